# revision 6
# baseline (speedup 1.0000x reference)
"""MultiHeadAttention Trainium2 kernel.

Sharding: 8 cores = 2 batches x 4 head-groups (3 heads each).
Each core computes, for its batch b and heads hs = [3g, 3g+1, 3g+2]:
    partial_out = sum_h softmax(mask( (q Wq_h)(k Wk_h)^T / 8 )) (v Wv_h) @ Wo[h]
Host sums the 4 partials per batch.

Device pipeline (all matmul operands bf16, PSUM f32):
  - inputs pre-transposed on host: qT,kT,vT [768,2048], maskT [2048,2048] (0/1 bf16)
  - projections: qhT/khT [64,2048] per head (heads A,B packed in one [128,2048]
    tile), vh [128 t-chunk, 3*65] with a constant-1 column per head
  - scores computed transposed: scoresT[t,s] = sum_k khT[k,t] qhT[k,s]
  - exp on ScalarE with scale=1/8 folded in (no max subtraction: |scores/8|<~7)
  - mask multiply on VectorE (bf16 0/1)
  - PV: outT_aug[k,s] = sum_t vh_aug[t,k] P[t,s]; row 64 = softmax denominator
  - denominators: reciprocal in [128,16] layout via DRAM bounce, broadcast back
  - Wo: out[s,:] = sum_h (outT_h/denom_h)^T @ Wo_h
"""

import os
import numpy as np
import ml_dtypes
from contextlib import ExitStack

B = 2
S = 2048
D = 768
H = 12
DK = 64
HPC = 3  # heads per core
NCORES = 8
NG = 4  # head groups per batch
DCH = 6  # d_model chunks of 128
TCH = 16  # t chunks of 128
SCH = 4  # s chunks of 512

_CACHE = {}
LAST_RESULT = None


def _build_bass():
    import concourse.bass as bass
    import concourse.tile as tile
    import concourse.mybir as mybir
    from concourse import bacc

    bf16 = mybir.dt.bfloat16
    f32 = mybir.dt.float32
    Alu = mybir.AluOpType
    Act = mybir.ActivationFunctionType

    nc = bacc.Bacc("TRN2", target_bir_lowering=False, debug=False, num_devices=NCORES)

    qT = nc.dram_tensor("qT", [D, S], bf16, kind="ExternalInput").ap()
    kT = nc.dram_tensor("kT", [D, S], bf16, kind="ExternalInput").ap()
    vT = nc.dram_tensor("vT", [D, S], bf16, kind="ExternalInput").ap()
    maskT = nc.dram_tensor("maskT", [S, S], bf16, kind="ExternalInput").ap()
    wq = nc.dram_tensor("wq", [D, HPC * DK], bf16, kind="ExternalInput").ap()
    wk = nc.dram_tensor("wk", [D, HPC * DK], bf16, kind="ExternalInput").ap()
    wv = nc.dram_tensor("wv", [D, HPC * DK], bf16, kind="ExternalInput").ap()
    wo = nc.dram_tensor("wo", [HPC * DK, D], bf16, kind="ExternalInput").ap()
    outp = nc.dram_tensor("outp", [S, D], f32, kind="ExternalOutput").ap()

    with tile.TileContext(nc) as tc:
        with ExitStack() as ctx:
            # ------- persistent pools -------
            p_per = ctx.enter_context(tc.tile_pool(name="p_per", bufs=1))
            p_small = ctx.enter_context(tc.tile_pool(name="p_small", bufs=1))
            p_psum = ctx.enter_context(tc.tile_pool(name="p_psum", bufs=1, space="PSUM"))
            p_dram = ctx.enter_context(tc.tile_pool(name="p_dram", bufs=2, space="DRAM"))

            mask_sb = p_per.tile([128, TCH, S], bf16, tag="mask")
            maskT_r = maskT.rearrange("(c p) s -> c p s", p=128)
            for t in range(TCH):
                nc.sync.dma_start(mask_sb[:, t, :], maskT_r[t])

            wq_sb = p_per.tile([128, DCH, HPC * DK], bf16, tag="wq")
            wk_sb = p_per.tile([128, DCH, HPC * DK], bf16, tag="wk")
            wv_sb = p_per.tile([128, DCH, HPC * DK], bf16, tag="wv")
            nc.sync.dma_start(wq_sb[:], wq.rearrange("(c p) m -> p c m", p=128))
            nc.sync.dma_start(wk_sb[:], wk.rearrange("(c p) m -> p c m", p=128))
            nc.sync.dma_start(wv_sb[:], wv.rearrange("(c p) m -> p c m", p=128))

            # qhT_ab rows 0-63 = head0, rows 64-127 = head1; qhT_c = head2
            qhT_ab = p_per.tile([128, S], bf16, tag="qhT_ab")
            khT_ab = p_per.tile([128, S], bf16, tag="khT_ab")
            qhT_c = p_per.tile([64, S], bf16, tag="qhT_c")
            khT_c = p_per.tile([64, S], bf16, tag="khT_c")
            # vh: [128 t, 3*65], col h*65+64 is the ones column
            vh_sb = p_per.tile([128, TCH, HPC * 65], bf16, tag="vh")

            # ------- phase 1a: q/k projections -------
            with tc.tile_pool(name="p_in_qk", bufs=1) as p_in:
                qT_sb = p_in.tile([128, DCH, S], bf16, tag="qT")
                kT_sb = p_in.tile([128, DCH, S], bf16, tag="kT")
                nc.sync.dma_start(qT_sb[:], qT.rearrange("(c p) s -> p c s", p=128))
                nc.sync.dma_start(kT_sb[:], kT.rearrange("(c p) s -> p c s", p=128))

                for (dst, w_sb, src, mlo, mhi) in (
                    (qhT_ab, wq_sb, qT_sb, 0, 128),
                    (khT_ab, wk_sb, kT_sb, 0, 128),
                    (qhT_c, wq_sb, qT_sb, 128, 192),
                    (khT_c, wk_sb, kT_sb, 128, 192),
                ):
                    mw = mhi - mlo
                    for sc in range(SCH):
                        ps = p_psum.tile([128, 512], f32, tag="scores", name="ps_proj")
                        for c in range(DCH):
                            nc.tensor.matmul(
                                ps[:mw, :],
                                w_sb[:, c, mlo:mhi],
                                src[:, c, sc * 512:(sc + 1) * 512],
                                start=(c == 0),
                                stop=(c == DCH - 1),
                            )
                        nc.vector.tensor_copy(
                            dst[:, sc * 512:(sc + 1) * 512], ps[:mw, :]
                        )

            # ------- phase 1b: v projection -------
            with tc.tile_pool(name="p_in_v", bufs=1) as p_in:
                vT_sb = p_in.tile([128, DCH, S], bf16, tag="vT")
                nc.sync.dma_start(vT_sb[:], vT.rearrange("(c p) s -> p c s", p=128))
                for h in range(HPC):
                    nc.vector.memset(vh_sb[:, :, h * 65 + 64], 1.0)
                for t in range(TCH):
                    ps = p_psum.tile([128, HPC * DK], f32, tag="pv", name="ps_v")
                    for c in range(DCH):
                        nc.tensor.matmul(
                            ps[:],
                            vT_sb[:, c, t * 128:(t + 1) * 128],
                            wv_sb[:, c, :],
                            start=(c == 0),
                            stop=(c == DCH - 1),
                        )
                    nc.vector.tensor_copy(
                        vh_sb[:, t, :].rearrange("p (h x) -> p h x", x=65)[:, :, 0:DK],
                        ps.rearrange("p (h x) -> p h x", x=DK),
                    )

            # ------- phase 2: attention -------
            with tc.tile_pool(name="p_work", bufs=1) as p_work:
                wo_sb = [
                    p_work.tile([64, D], bf16, tag=f"wo{h}", name=f"wo{h}")
                    for h in range(HPC)
                ]
                for h in range(HPC):
                    nc.sync.dma_start(wo_sb[h][:], wo[h * 64:(h + 1) * 64, :])

                outT = []  # scaled head outputs [64, S] bf16
                for h in range(HPC):
                    if h == 0:
                        lhs_k, rhs_q, plo = khT_ab, qhT_ab, 0
                    elif h == 1:
                        lhs_k, rhs_q, plo = khT_ab, qhT_ab, 64
                    else:
                        lhs_k, rhs_q, plo = khT_c, qhT_c, 0
                    phi = plo + 64

                    ps_o = p_psum.tile([65, S], f32, tag="pv", name="ps_o")
                    for t in range(TCH):
                        ps_s = p_psum.tile([128, S], f32, tag="scores", name="ps_s")
                        for sc in range(SCH):
                            nc.tensor.matmul(
                                ps_s[:, sc * 512:(sc + 1) * 512],
                                lhs_k[plo:phi, t * 128:(t + 1) * 128],
                                rhs_q[plo:phi, sc * 512:(sc + 1) * 512],
                                start=True,
                                stop=True,
                            )
                        pt = p_work.tile([128, S], bf16, tag="pt", name="pt", bufs=2)
                        nc.scalar.activation(pt[:], ps_s[:], Act.Exp, scale=0.125)
                        nc.vector.tensor_tensor(
                            pt[:], pt[:], mask_sb[:, t, :], Alu.mult
                        )
                        for sc in range(SCH):
                            nc.tensor.matmul(
                                ps_o[:, sc * 512:(sc + 1) * 512],
                                vh_sb[:, t, h * 65:(h + 1) * 65],
                                pt[:, sc * 512:(sc + 1) * 512],
                                start=(t == 0),
                                stop=(t == TCH - 1),
                            )

                    # unnormalized head output + denominators
                    o_raw = p_work.tile([64, S], bf16, tag=f"oraw{h}", name=f"oraw{h}")
                    nc.vector.tensor_copy(o_raw[:], ps_o[0:64, :])
                    den = p_small.tile([1, S], f32, tag="denrow", name="den")
                    nc.vector.tensor_copy(den[:], ps_o[64:65, :])

                    # 1/den computed in [128,16] layout via DRAM bounce
                    dscr = p_dram.tile([S], f32, tag="dscr", name="dscr")
                    nc.sync.dma_start(dscr.unsqueeze(0), den[0:1, :])
                    denT = p_small.tile([128, TCH], f32, tag="denT", name="denT")
                    nc.sync.dma_start(denT[:], dscr.rearrange("(c p) -> p c", p=128))
                    rdenT32 = p_small.tile([128, TCH], f32, tag="rdenT32", name="rdenT32")
                    nc.vector.reciprocal(rdenT32[:], denT[:])
                    rdenT = p_small.tile([128, TCH], bf16, tag="rdenT", name="rdenT")
                    nc.vector.tensor_copy(rdenT[:], rdenT32[:])
                    rscr = p_dram.tile([S], bf16, tag="rscr", name="rscr")
                    nc.sync.dma_start(rscr.rearrange("(c p) -> p c", p=128), rdenT[:])
                    rrow = p_small.tile([1, S], bf16, tag="denrow", name="rrow")
                    nc.sync.dma_start(rrow[:], rscr.unsqueeze(0))
                    rbc = p_work.tile([64, S], bf16, tag="rbc", name="rbc", bufs=2)
                    nc.gpsimd.partition_broadcast(rbc[:], rrow[0:1, :])
                    nc.vector.tensor_tensor(o_raw[:], o_raw[:], rbc[:], Alu.mult)
                    outT.append(o_raw)

                # ------- output projection -------
                for s in range(TCH):
                    ps_w = p_psum.tile(
                        [128, D], f32,
                        tag="scores" if s % 2 == 0 else "pv", name="ps_w",
                    )
                    for nlo, nhi in ((0, 512), (512, D)):
                        for h in range(HPC):
                            nc.tensor.matmul(
                                ps_w[:, nlo:nhi],
                                outT[h][:, s * 128:(s + 1) * 128],
                                wo_sb[h][:, nlo:nhi],
                                start=(h == 0),
                                stop=(h == HPC - 1),
                            )
                    o_sb = p_work.tile([128, D], f32, tag="osb", name="osb", bufs=2)
                    nc.vector.tensor_copy(o_sb[:], ps_w[:])
                    nc.sync.dma_start(outp[s * 128:(s + 1) * 128, :], o_sb[:])

    nc.compile()
    return nc


def _get_nc():
    if "nc" not in _CACHE:
        _CACHE["nc"] = _build_bass()
    return _CACHE["nc"]


def kernel(q, k, v, mask, Wq, Wk, Wv, Wo, trace=False):
    global LAST_RESULT
    from concourse.bass_utils import run_bass_kernel_spmd

    bf16 = ml_dtypes.bfloat16
    q = np.asarray(q)
    k = np.asarray(k)
    v = np.asarray(v)
    mask = np.asarray(mask)
    Wq = np.asarray(Wq)
    Wk = np.asarray(Wk)
    Wv = np.asarray(Wv)
    Wo = np.asarray(Wo)

    per_b = []
    for b in range(B):
        per_b.append(
            dict(
                qT=np.ascontiguousarray(q[b].T).astype(bf16),
                kT=np.ascontiguousarray(k[b].T).astype(bf16),
                vT=np.ascontiguousarray(v[b].T).astype(bf16),
                maskT=np.ascontiguousarray(mask[b, 0].T).astype(bf16),
            )
        )
    per_g = []
    for g in range(NG):
        hs = range(g * HPC, (g + 1) * HPC)
        per_g.append(
            dict(
                wq=np.concatenate([Wq[h] for h in hs], axis=1).astype(bf16),
                wk=np.concatenate([Wk[h] for h in hs], axis=1).astype(bf16),
                wv=np.concatenate([Wv[h] for h in hs], axis=1).astype(bf16),
                wo=Wo[g * HPC * DK:(g + 1) * HPC * DK, :].astype(bf16),
            )
        )

    in_maps = []
    for core in range(NCORES):
        b, g = divmod(core, NG)
        m = {}
        m.update(per_b[b])
        m.update(per_g[g])
        in_maps.append(m)

    nc = _get_nc()
    res = run_bass_kernel_spmd(nc, in_maps, list(range(NCORES)), trace=trace)
    LAST_RESULT = res

    out = np.zeros((B, S, D), dtype=np.float32)
    for core in range(NCORES):
        b = core // NG
        out[b] += res.results[core]["outp"]
    return out


# revision 7
# speedup vs baseline: 1.1354x; 1.1354x over previous
"""MultiHeadAttention Trainium2 kernel.

Sharding: 8 cores = 2 batches x 4 head-groups (3 heads each).
Each core computes, for its batch b and heads hs = [3g, 3g+1, 3g+2]:
    partial_out = sum_h softmax(mask( (q Wq_h)(k Wk_h)^T / 8 )) (v Wv_h) @ Wo[h]
Host sums the 4 partials per batch.

Device pipeline (all matmul operands bf16, PSUM f32):
  - inputs pre-transposed on host: qT,kT,vT [768,2048], maskT [2048,2048] (0/1 bf16)
  - projections: qhT/khT [64,2048] per head (heads A,B packed in one [128,2048]
    tile), vh [128 t-chunk, 3*65] with a constant-1 column per head
  - scores computed transposed: scoresT[t,s] = sum_k khT[k,t] qhT[k,s]
  - exp on ScalarE with scale=1/8 folded in (no max subtraction: |scores/8|<~7)
  - mask multiply on VectorE (bf16 0/1)
  - PV: outT_aug[k,s] = sum_t vh_aug[t,k] P[t,s]; row 64 = softmax denominator
  - denominators: reciprocal in [128,16] layout via DRAM bounce, broadcast back
  - Wo: out[s,:] = sum_h (outT_h/denom_h)^T @ Wo_h
"""

import os
import numpy as np
import ml_dtypes
from contextlib import ExitStack

B = 2
S = 2048
D = 768
H = 12
DK = 64
HPC = 3  # heads per core
NCORES = 8
NG = 4  # head groups per batch
DCH = 6  # d_model chunks of 128
TCH = 16  # t chunks of 128
SCH = 4  # s chunks of 512

_CACHE = {}
LAST_RESULT = None


def _build_bass():
    import concourse.bass as bass
    import concourse.tile as tile
    import concourse.mybir as mybir
    from concourse import bacc

    bf16 = mybir.dt.bfloat16
    f32 = mybir.dt.float32
    Alu = mybir.AluOpType
    Act = mybir.ActivationFunctionType

    nc = bacc.Bacc("TRN2", target_bir_lowering=False, debug=False, num_devices=NCORES)

    qT = nc.dram_tensor("qT", [D, S], bf16, kind="ExternalInput").ap()
    kT = nc.dram_tensor("kT", [D, S], bf16, kind="ExternalInput").ap()
    vT = nc.dram_tensor("vT", [D, S], bf16, kind="ExternalInput").ap()
    maskT = nc.dram_tensor("maskT", [S, S], bf16, kind="ExternalInput").ap()
    wq = nc.dram_tensor("wq", [D, HPC * DK], bf16, kind="ExternalInput").ap()
    wk = nc.dram_tensor("wk", [D, HPC * DK], bf16, kind="ExternalInput").ap()
    wv = nc.dram_tensor("wv", [D, HPC * DK], bf16, kind="ExternalInput").ap()
    wo = nc.dram_tensor("wo", [HPC * DK, D], bf16, kind="ExternalInput").ap()
    outp = nc.dram_tensor("outp", [S, D], f32, kind="ExternalOutput").ap()

    with tile.TileContext(nc) as tc:
        with ExitStack() as ctx:
            # ------- persistent pools -------
            p_per = ctx.enter_context(tc.tile_pool(name="p_per", bufs=1))
            p_small = ctx.enter_context(tc.tile_pool(name="p_small", bufs=1))
            p_psum = ctx.enter_context(tc.tile_pool(name="p_psum", bufs=1, space="PSUM"))
            p_dram = ctx.enter_context(tc.tile_pool(name="p_dram", bufs=2, space="DRAM"))

            mask_sb = p_per.tile([128, TCH, S], bf16, tag="mask")
            maskT_r = maskT.rearrange("(c p) s -> c p s", p=128)
            for t in range(TCH):
                nc.sync.dma_start(mask_sb[:, t, :], maskT_r[t])

            wq_sb = p_per.tile([128, DCH, HPC * DK], bf16, tag="wq")
            wk_sb = p_per.tile([128, DCH, HPC * DK], bf16, tag="wk")
            wv_sb = p_per.tile([128, DCH, HPC * DK], bf16, tag="wv")
            nc.sync.dma_start(wq_sb[:], wq.rearrange("(c p) m -> p c m", p=128))
            nc.sync.dma_start(wk_sb[:], wk.rearrange("(c p) m -> p c m", p=128))
            nc.sync.dma_start(wv_sb[:], wv.rearrange("(c p) m -> p c m", p=128))

            # qhT_ab rows 0-63 = head0, rows 64-127 = head1; qhT_c = head2
            qhT_ab = p_per.tile([128, S], bf16, tag="qhT_ab")
            khT_ab = p_per.tile([128, S], bf16, tag="khT_ab")
            qhT_c = p_per.tile([64, S], bf16, tag="qhT_c")
            khT_c = p_per.tile([64, S], bf16, tag="khT_c")
            # vh: [128 t, 3*65], col h*65+64 is the ones column
            vh_sb = p_per.tile([128, TCH, HPC * 65], bf16, tag="vh")

            # ------- phase 1a: q/k projections -------
            with tc.tile_pool(name="p_in_qk", bufs=1) as p_in:
                qT_sb = p_in.tile([128, DCH, S], bf16, tag="qT")
                kT_sb = p_in.tile([128, DCH, S], bf16, tag="kT")
                nc.sync.dma_start(qT_sb[:], qT.rearrange("(c p) s -> p c s", p=128))
                nc.sync.dma_start(kT_sb[:], kT.rearrange("(c p) s -> p c s", p=128))

                for (dst, w_sb, src, mlo, mhi) in (
                    (qhT_ab, wq_sb, qT_sb, 0, 128),
                    (khT_ab, wk_sb, kT_sb, 0, 128),
                    (qhT_c, wq_sb, qT_sb, 128, 192),
                    (khT_c, wk_sb, kT_sb, 128, 192),
                ):
                    mw = mhi - mlo
                    for sc in range(SCH):
                        ps = p_psum.tile([128, 512], f32, tag="sc0", name="ps_proj")
                        for c in range(DCH):
                            nc.tensor.matmul(
                                ps[:mw, :],
                                w_sb[:, c, mlo:mhi],
                                src[:, c, sc * 512:(sc + 1) * 512],
                                start=(c == 0),
                                stop=(c == DCH - 1),
                            )
                        nc.vector.tensor_copy(
                            dst[:, sc * 512:(sc + 1) * 512], ps[:mw, :]
                        )

            # ------- phase 1b: v projection -------
            with tc.tile_pool(name="p_in_v", bufs=1) as p_in:
                vT_sb = p_in.tile([128, DCH, S], bf16, tag="vT")
                nc.sync.dma_start(vT_sb[:], vT.rearrange("(c p) s -> p c s", p=128))
                for h in range(HPC):
                    nc.vector.memset(vh_sb[:, :, h * 65 + 64], 1.0)
                for t in range(TCH):
                    ps = p_psum.tile([128, HPC * DK], f32, tag="pv", name="ps_v")
                    for c in range(DCH):
                        nc.tensor.matmul(
                            ps[:],
                            vT_sb[:, c, t * 128:(t + 1) * 128],
                            wv_sb[:, c, :],
                            start=(c == 0),
                            stop=(c == DCH - 1),
                        )
                    nc.vector.tensor_copy(
                        vh_sb[:, t, :].rearrange("p (h x) -> p h x", x=65)[:, :, 0:DK],
                        ps.rearrange("p (h x) -> p h x", x=DK),
                    )

            # ------- phase 2: attention -------
            with tc.tile_pool(name="p_work", bufs=1) as p_work:
                wo_sb = [
                    p_work.tile([64, D], bf16, tag=f"wo{h}", name=f"wo{h}")
                    for h in range(HPC)
                ]
                for h in range(HPC):
                    nc.sync.dma_start(wo_sb[h][:], wo[h * 64:(h + 1) * 64, :])

                outT = []  # scaled head outputs [64, S] bf16
                for h in range(HPC):
                    if h == 0:
                        lhs_k, rhs_q, plo = khT_ab, qhT_ab, 0
                    elif h == 1:
                        lhs_k, rhs_q, plo = khT_ab, qhT_ab, 64
                    else:
                        lhs_k, rhs_q, plo = khT_c, qhT_c, 0
                    phi = plo + 64

                    ps_o = p_psum.tile([65, S], f32, tag="pv", name="ps_o")
                    for t in range(TCH):
                        # two half-width score slots so exp(half0) overlaps
                        # the next half's score matmuls on PE
                        for hf in range(2):
                            ps_s = p_psum.tile(
                                [128, 1024], f32, tag=f"sc{hf}", name=f"ps_s{hf}"
                            )
                            for sc in range(2):
                                off = hf * 1024 + sc * 512
                                nc.tensor.matmul(
                                    ps_s[:, sc * 512:(sc + 1) * 512],
                                    lhs_k[plo:phi, t * 128:(t + 1) * 128],
                                    rhs_q[plo:phi, off:off + 512],
                                    start=True,
                                    stop=True,
                                )
                            pt = p_work.tile(
                                [128, 1024], bf16, tag="pt", name="pt", bufs=3
                            )
                            nc.scalar.activation(pt[:], ps_s[:], Act.Exp, scale=0.125)
                            nc.vector.tensor_tensor(
                                pt[:],
                                pt[:],
                                mask_sb[:, t, hf * 1024:(hf + 1) * 1024],
                                Alu.mult,
                            )
                            for sc in range(2):
                                off = hf * 1024 + sc * 512
                                nc.tensor.matmul(
                                    ps_o[:, off:off + 512],
                                    vh_sb[:, t, h * 65:(h + 1) * 65],
                                    pt[:, sc * 512:(sc + 1) * 512],
                                    start=(t == 0),
                                    stop=(t == TCH - 1),
                                )

                    # unnormalized head output + denominators
                    o_raw = p_work.tile([64, S], bf16, tag=f"oraw{h}", name=f"oraw{h}")
                    nc.vector.tensor_copy(o_raw[:], ps_o[0:64, :])
                    den = p_small.tile([1, S], f32, tag="denrow", name="den")
                    nc.vector.tensor_copy(den[:], ps_o[64:65, :])

                    # 1/den computed in [128,16] layout via DRAM bounce
                    dscr = p_dram.tile([S], f32, tag="dscr", name="dscr")
                    nc.sync.dma_start(dscr.unsqueeze(0), den[0:1, :])
                    denT = p_small.tile([128, TCH], f32, tag="denT", name="denT")
                    nc.sync.dma_start(denT[:], dscr.rearrange("(c p) -> p c", p=128))
                    rdenT32 = p_small.tile([128, TCH], f32, tag="rdenT32", name="rdenT32")
                    nc.vector.reciprocal(rdenT32[:], denT[:])
                    rdenT = p_small.tile([128, TCH], bf16, tag="rdenT", name="rdenT")
                    nc.vector.tensor_copy(rdenT[:], rdenT32[:])
                    rscr = p_dram.tile([S], bf16, tag="rscr", name="rscr")
                    nc.sync.dma_start(rscr.rearrange("(c p) -> p c", p=128), rdenT[:])
                    rrow = p_small.tile([1, S], bf16, tag="denrow", name="rrow")
                    nc.sync.dma_start(rrow[:], rscr.unsqueeze(0))
                    rbc = p_work.tile([64, S], bf16, tag="rbc", name="rbc", bufs=2)
                    nc.gpsimd.partition_broadcast(rbc[:], rrow[0:1, :])
                    nc.vector.tensor_tensor(o_raw[:], o_raw[:], rbc[:], Alu.mult)
                    outT.append(o_raw)

                # ------- output projection -------
                for s in range(TCH):
                    ps_w = p_psum.tile(
                        [128, D], f32,
                        tag="sc0" if s % 2 == 0 else "sc1", name="ps_w",
                    )
                    for nlo, nhi in ((0, 512), (512, D)):
                        for h in range(HPC):
                            nc.tensor.matmul(
                                ps_w[:, nlo:nhi],
                                outT[h][:, s * 128:(s + 1) * 128],
                                wo_sb[h][:, nlo:nhi],
                                start=(h == 0),
                                stop=(h == HPC - 1),
                            )
                    o_sb = p_work.tile([128, D], f32, tag="osb", name="osb", bufs=2)
                    nc.vector.tensor_copy(o_sb[:], ps_w[:])
                    nc.sync.dma_start(outp[s * 128:(s + 1) * 128, :], o_sb[:])

    nc.compile()
    return nc


def _get_nc():
    if "nc" not in _CACHE:
        _CACHE["nc"] = _build_bass()
    return _CACHE["nc"]


def kernel(q, k, v, mask, Wq, Wk, Wv, Wo, trace=False):
    global LAST_RESULT
    from concourse.bass_utils import run_bass_kernel_spmd

    bf16 = ml_dtypes.bfloat16
    q = np.asarray(q)
    k = np.asarray(k)
    v = np.asarray(v)
    mask = np.asarray(mask)
    Wq = np.asarray(Wq)
    Wk = np.asarray(Wk)
    Wv = np.asarray(Wv)
    Wo = np.asarray(Wo)

    per_b = []
    for b in range(B):
        per_b.append(
            dict(
                qT=np.ascontiguousarray(q[b].T).astype(bf16),
                kT=np.ascontiguousarray(k[b].T).astype(bf16),
                vT=np.ascontiguousarray(v[b].T).astype(bf16),
                maskT=np.ascontiguousarray(mask[b, 0].T).astype(bf16),
            )
        )
    per_g = []
    for g in range(NG):
        hs = range(g * HPC, (g + 1) * HPC)
        per_g.append(
            dict(
                wq=np.concatenate([Wq[h] for h in hs], axis=1).astype(bf16),
                wk=np.concatenate([Wk[h] for h in hs], axis=1).astype(bf16),
                wv=np.concatenate([Wv[h] for h in hs], axis=1).astype(bf16),
                wo=Wo[g * HPC * DK:(g + 1) * HPC * DK, :].astype(bf16),
            )
        )

    in_maps = []
    for core in range(NCORES):
        b, g = divmod(core, NG)
        m = {}
        m.update(per_b[b])
        m.update(per_g[g])
        in_maps.append(m)

    nc = _get_nc()
    res = run_bass_kernel_spmd(nc, in_maps, list(range(NCORES)), trace=trace)
    LAST_RESULT = res

    out = np.zeros((B, S, D), dtype=np.float32)
    for core in range(NCORES):
        b = core // NG
        out[b] += res.results[core]["outp"]
    return out
